# revision 14
# baseline (speedup 1.0000x reference)
"""GNN message-passing (2-layer conv + log_softmax) as a Bass/Tile SPMD kernel
on 8 Trainium2 NeuronCores.

Strategy (dst-sharded 1D graph partition, replicated message tables):
  - nodes sharded 8-way; core k owns dst nodes [k*NP, (k+1)*NP)
  - L1: h1 = x@W1 + b1 computed on node shards (host-pretransposed bf16 xT),
    chunk-wise AllGather -> full bf16 table tb1 (chunk-major row layout)
  - aggregation: per-core dsts sorted by in-degree, grouped into 128-dst
    windows padded to the window max degree; messages fetched with indirect
    DMA gathers (row per edge slot, pad slots hit a zero row) and reduced on
    the TensorEngine by identity-matmul PSUM accumulation (exact fp32)
  - elu folded as g' = relu(f) + exp(min(f,0)) = elu(f)+1, compensated by
    passing b2' = b2 - W2.sum(0); t2 = g'@W2 + b2' built per window (PE
    transpose + matmul), AllGather#2 -> table tb2; second gather+reduce;
    log_softmax fused on ACT/DVE. Output rows are in per-core degree-perm
    order; the host inverts the permutation.
"""

import os
import sys

sys.path.insert(0, "/opt/trn_rl_repo")

import numpy as np
import ml_dtypes

BF16 = ml_dtypes.bfloat16

# static problem config (full-size); tests may build their own cfg
N_CORES = 8
P = 128


def _make_cfg(n_nodes, n_edges, f_in=512, hid=64, n_cls=40, ctarget=256):
    np_ = n_nodes // N_CORES
    assert np_ * N_CORES == n_nodes
    nw = (np_ + P - 1) // P
    npad = nw * P
    n_chunks = min(4, nw)
    # chunk boundaries in units of 128-row tiles; the LAST chunk is kept small
    # (its AllGather is the only one that cannot overlap compute, so it sets
    # the phase-boundary stall)
    if nw >= 16:
        last = max(1, round(nw * 0.08))
        rest = nw - last
        tiles = [rest // 3 + (1 if i < rest % 3 else 0) for i in range(3)] + [last]
    else:
        tiles = [nw // n_chunks + (1 if i < nw % n_chunks else 0)
                 for i in range(n_chunks)]
    tstart = np.concatenate([[0], np.cumsum(tiles)])
    # table1 chunks cover real local rows [t0*128, min(t1*128, np_))
    c1_start = [int(min(tstart[i] * P, np_)) for i in range(n_chunks + 1)]
    c1_size = [c1_start[i + 1] - c1_start[i] for i in range(n_chunks)]
    # table2 chunks cover padded rows [t0*128, t1*128)
    c2_start = [int(tstart[i] * P) for i in range(n_chunks + 1)]
    c2_size = [c2_start[i + 1] - c2_start[i] for i in range(n_chunks)]
    base1 = np.concatenate([[0], np.cumsum([N_CORES * s for s in c1_size])])
    base2 = np.concatenate([[0], np.cumsum([N_CORES * s for s in c2_size])])
    tot1 = int(base1[-1])  # == n_nodes
    tot2 = int(base2[-1])  # == 8 * npad
    return dict(
        N=n_nodes, E=n_edges, F=f_in, H=hid, C=n_cls, NP=np_, NW=nw, NPAD=npad,
        NCH=n_chunks, TILES=tiles, TSTART=tstart,
        C1S=c1_start, C1Z=c1_size, C2S=c2_start, C2Z=c2_size,
        BASE1=base1, BASE2=base2, TOT1=tot1, TOT2=tot2,
        ZROW1=tot1, ZROW2=tot2, CTARGET=ctarget,
    )


FULL_CFG = _make_cfg(100000, 3200000)


# ---------------------------------------------------------------- host prep

def _row_maps(cfg, pos_all):
    """map global node id -> table1 row / table2 row (chunk-major layouts).
    pos_all: [N] position of each node within its core's degree-perm."""
    N, NP = cfg["N"], cfg["NP"]
    g = np.arange(N, dtype=np.int64)
    r = g // NP
    l = g % NP
    c1b = np.asarray(cfg["C1S"])
    c = np.searchsorted(c1b, l, side="right") - 1
    sz = np.asarray(cfg["C1Z"] + [1])[c]
    map1 = np.asarray(cfg["BASE1"])[c] + r * sz + (l - c1b[c])
    p = pos_all
    c2b = np.asarray(cfg["C2S"])
    c2 = np.searchsorted(c2b, p, side="right") - 1
    sz2 = np.asarray(cfg["C2Z"] + [1])[c2]
    map2 = np.asarray(cfg["BASE2"])[c2] + r * sz2 + (p - c2b[c2])
    map1 = np.concatenate([map1, [cfg["ZROW1"]]]).astype(np.int32)
    map2 = np.concatenate([map2, [cfg["ZROW2"]]]).astype(np.int32)
    return map1, map2


def host_prep(cfg, x, edge_index, W1, b1, W2, b2):
    N, NP, NW = cfg["N"], cfg["NP"], cfg["NW"]
    src = np.asarray(edge_index[0]).astype(np.int64)
    dst = np.asarray(edge_index[1]).astype(np.int64)

    per_core = []
    for k in range(N_CORES):
        sel = (dst >= k * NP) & (dst < (k + 1) * NP)
        s_k = src[sel]
        d_k = (dst[sel] - k * NP).astype(np.int64)
        deg = np.bincount(d_k, minlength=NP)
        perm = np.argsort(-deg, kind="stable")
        pos = np.empty(NP, dtype=np.int64)
        pos[perm] = np.arange(NP)
        order = np.argsort(d_k, kind="stable")
        ss = s_k[order]
        starts = np.concatenate([[0], np.cumsum(deg)])
        per_core.append(dict(deg=deg, perm=perm, pos=pos, ss=ss, starts=starts))

    # window capacities (uniform across cores)
    D = np.zeros(NW, dtype=np.int64)
    for k in range(N_CORES):
        deg, perm = per_core[k]["deg"], per_core[k]["perm"]
        for w in range(NW):
            n0 = perm[w * P] if w * P < NP else None
            dw = int(deg[n0]) if n0 is not None else 0
            D[w] = max(D[w], dw)
    D = np.maximum(D, 1)

    # greedy grouping of windows into gather calls
    groups = []  # (list of w, list of D_w, colstart)
    cur, curD = [], 0
    for w in range(NW):
        if cur and curD + D[w] > cfg["CTARGET"]:
            groups.append((cur, curD))
            cur, curD = [], 0
        cur.append(w)
        curD += int(D[w])
    if cur:
        groups.append((cur, curD))
    woff = np.concatenate([[0], np.cumsum(D)])  # col offset per window
    sumc = int(woff[-1])

    # raw src blocks per core (sentinel N for padding), then remap
    pos_all = np.concatenate([pc["pos"] for pc in per_core])
    map1, map2 = _row_maps(cfg, pos_all)
    idx1, idx2 = [], []
    for k in range(N_CORES):
        pc = per_core[k]
        raw = np.full((P, sumc), N, dtype=np.int64)
        deg, perm, ss, starts = pc["deg"], pc["perm"], pc["ss"], pc["starts"]
        for w in range(NW):
            for p in range(min(P, NP - w * P)):
                n = perm[w * P + p]
                dn = deg[n]
                if dn:
                    raw[p, woff[w]:woff[w] + dn] = ss[starts[n]:starts[n] + dn]
        idx1.append(map1[raw])
        idx2.append(map2[raw])

    # per-core tensors
    W1b = np.asarray(W1, dtype=np.float32).astype(BF16)
    W2b = np.asarray(W2, dtype=np.float32).astype(BF16)
    b1r = np.tile(np.asarray(b1, dtype=np.float32)[None, :], (P, 1))
    b2a = np.asarray(b2, dtype=np.float32) - np.asarray(W2, np.float32).sum(0)
    b2r = np.tile(b2a[None, :], (P, 1))
    in_maps = []
    xf = np.asarray(x, dtype=np.float32)
    for k in range(N_CORES):
        xT = np.ascontiguousarray(xf[k * NP:(k + 1) * NP].T).astype(BF16)
        in_maps.append(dict(
            xT=xT, W1=W1b, b1r=b1r, W2=W2b, b2r=b2r,
            idx1=idx1[k], idx2=idx2[k],
        ))
    sched = dict(D=D, groups=groups, woff=woff, sumc=sumc)
    perms = [pc["perm"] for pc in per_core]
    return sched, in_maps, perms


# ---------------------------------------------------------------- device code

def build_program(cfg, sched):
    import concourse.bass as bass
    import concourse.bacc as bacc
    import concourse.mybir as mybir
    from concourse.tile import TileContext
    from concourse.masks import make_identity

    dt = mybir.dt
    N, F, H, C = cfg["N"], cfg["F"], cfg["H"], cfg["C"]
    NP, NW, NPAD, NCH = cfg["NP"], cfg["NW"], cfg["NPAD"], cfg["NCH"]
    D, groups, woff, sumc = sched["D"], sched["groups"], sched["woff"], sched["sumc"]
    KF = F // P

    nc = bacc.Bacc(
        "TRN2", target_bir_lowering=False, debug=False, num_devices=N_CORES
    )
    xT = nc.declare_dram_parameter("xT", [F, NP], dt.bfloat16, isOutput=False)
    W1p = nc.declare_dram_parameter("W1", [F, H], dt.bfloat16, isOutput=False)
    b1p = nc.declare_dram_parameter("b1r", [P, H], dt.float32, isOutput=False)
    W2p = nc.declare_dram_parameter("W2", [H, C], dt.bfloat16, isOutput=False)
    b2p = nc.declare_dram_parameter("b2r", [P, C], dt.float32, isOutput=False)
    ix1p = nc.declare_dram_parameter("idx1", [P, sumc], dt.int32, isOutput=False)
    ix2p = nc.declare_dram_parameter("idx2", [P, sumc], dt.int32, isOutput=False)
    outp = nc.declare_dram_parameter("out", [NPAD, C], dt.float32, isOutput=True)

    rg = [list(range(N_CORES))]
    cmax = max(cD for _, cD in groups)

    with TileContext(nc) as tc:
        with (
            tc.tile_pool(name="const", bufs=1) as const,
            tc.tile_pool(name="dram", bufs=1, space="DRAM") as dram,
            tc.tile_pool(name="xp", bufs=2) as xp,
            tc.tile_pool(name="hp", bufs=8) as hp,
            tc.tile_pool(name="ixp", bufs=1) as ixp,
            tc.tile_pool(name="gp", bufs=8) as gpl,
            tc.tile_pool(name="sp", bufs=3) as sp,
            tc.tile_pool(name="ps", bufs=2, space="PSUM") as ps,
        ):
            # --- constants
            w1sb = const.tile([P, KF, H], dt.bfloat16)
            nc.sync.dma_start(out=w1sb[:], in_=W1p[:].rearrange("(c p) h -> p c h", p=P))
            w2sb = const.tile([H, C], dt.bfloat16)
            nc.sync.dma_start(out=w2sb[:], in_=W2p[:])
            b1sb = const.tile([P, H], dt.float32)
            nc.sync.dma_start(out=b1sb[:], in_=b1p[:])
            b2sb = const.tile([P, C], dt.float32)
            nc.sync.dma_start(out=b2sb[:], in_=b2p[:])
            ident = const.tile([P, P], dt.bfloat16)
            make_identity(nc, ident[:])

            # --- internal DRAM
            h1k = dram.tile([NP, H], dt.bfloat16)
            t2k = dram.tile([NPAD, C], dt.bfloat16)
            tb1 = dram.tile([cfg["TOT1"] + 1, H], dt.bfloat16)
            tb2 = dram.tile([cfg["TOT2"] + 1, C], dt.bfloat16)

            # zero rows for padding slots
            zt = const.tile([1, H], dt.bfloat16)
            nc.gpsimd.memset(zt[:], 0.0)
            nc.sync.dma_start(out=tb1[cfg["ZROW1"]:cfg["ZROW1"] + 1, :], in_=zt[:, :H])
            nc.sync.dma_start(out=tb2[cfg["ZROW2"]:cfg["ZROW2"] + 1, :], in_=zt[:, :C])

            # --- resident index tables (one DMA each, not one per edge slot)
            ix1sb = ixp.tile([P, sumc], dt.int32, tag="ix1sb")
            nc.sync.dma_start(out=ix1sb[:], in_=ix1p[:])
            ix2sb = ixp.tile([P, sumc], dt.int32, tag="ix2sb")
            nc.sync.dma_start(out=ix2sb[:], in_=ix2p[:])

            # --- phase 1: h1 = x@W1 + b1 on local shard, chunked AllGather
            # xT staged per AG-chunk in one big DMA (not 98 small tile loads)
            xTr = xT[:].rearrange("(c p) n -> p c n", p=P)
            for ch in range(NCH):
                t0, t1 = int(cfg["TSTART"][ch]), int(cfg["TSTART"][ch + 1])
                n0, n1 = t0 * P, min(t1 * P, NP)
                if n1 <= n0:
                    continue
                xt = xp.tile([P, KF, (int(cfg["TSTART"][1]) - 0) * P], dt.bfloat16,
                             tag="xt")
                nc.sync.dma_start(out=xt[:, :, :n1 - n0], in_=xTr[:, :, n0:n1])
                for nt in range(t0, t1):
                    cs = min(P, NP - nt * P)
                    if cs <= 0:
                        continue
                    o0 = nt * P - n0
                    ph = ps.tile([P, H], dt.float32, tag="ph")
                    for kf in range(KF):
                        nc.tensor.matmul(
                            out=ph[:cs, :], lhsT=xt[:, kf, o0:o0 + cs],
                            rhs=w1sb[:, kf, :],
                            start=(kf == 0), stop=(kf == KF - 1),
                        )
                    h1sb = hp.tile([P, H], dt.bfloat16, tag="h1sb")
                    nc.vector.tensor_tensor(
                        out=h1sb[:cs, :], in0=ph[:cs, :], in1=b1sb[:cs, :],
                        op=mybir.AluOpType.add,
                    )
                    nc.sync.dma_start(out=h1k[nt * P:nt * P + cs, :], in_=h1sb[:cs, :])
                # gather this chunk of h1 across cores
                s0, sz = cfg["C1S"][ch], cfg["C1Z"][ch]
                nc.gpsimd.collective_compute(
                    "AllGather", mybir.AluOpType.bypass, replica_groups=rg,
                    ins=[h1k[s0:s0 + sz, :]],
                    outs=[tb1[int(cfg["BASE1"][ch]):int(cfg["BASE1"][ch]) + N_CORES * sz, :]],
                )

            # --- phase 2: L1 gather+reduce, elu', t2 rows, chunked AllGather#2
            ch_end = {int(cfg["TSTART"][ch + 1]) - 1: ch for ch in range(NCH)}
            for w in range(NW):
                dw = int(D[w])
                c0 = int(woff[w])
                gts = []
                for s in range(dw):
                    gt = gpl.tile([P, H], dt.bfloat16, tag="gt")
                    nc.gpsimd.indirect_dma_start(
                        out=gt[:], out_offset=None,
                        in_=tb1[:],
                        in_offset=bass.IndirectOffsetOnAxis(
                            ap=ix1sb[:, c0 + s:c0 + s + 1], axis=0),
                    )
                    gts.append(gt)
                red = ps.tile([P, H], dt.float32, tag="red")
                for s in range(dw):
                    nc.tensor.matmul(
                        out=red[:, :], lhsT=ident[:],
                        rhs=gts[s][:],
                        start=(s == 0), stop=(s == dw - 1),
                    )
                if True:
                    # g' = relu(f) + exp(min(f, 0))  (= elu(f) + 1)
                    m = sp.tile([P, H], dt.float32, tag="m")
                    nc.vector.tensor_scalar_min(out=m[:], in0=red[:], scalar1=0.0)
                    e = sp.tile([P, H], dt.float32, tag="e")
                    nc.scalar.activation(e[:], m[:], mybir.ActivationFunctionType.Exp)
                    gpr = sp.tile([P, H], dt.bfloat16, tag="gpr")
                    nc.vector.scalar_tensor_tensor(
                        out=gpr[:], in0=red[:], scalar=0.0, in1=e[:],
                        op0=mybir.AluOpType.max, op1=mybir.AluOpType.add,
                    )
                    # t2 row block = g'@W2 + b2'
                    tr = ps.tile([H, P], dt.bfloat16, tag="tr")
                    nc.tensor.transpose(out=tr[:], in_=gpr[:], identity=ident[:])
                    trsb = sp.tile([H, P], dt.bfloat16, tag="trsb")
                    nc.vector.tensor_copy(out=trsb[:], in_=tr[:])
                    t2p = ps.tile([P, C], dt.float32, tag="t2p")
                    nc.tensor.matmul(out=t2p[:], lhsT=trsb[:], rhs=w2sb[:],
                                     start=True, stop=True)
                    t2sb = sp.tile([P, C], dt.bfloat16, tag="t2sb")
                    nc.vector.tensor_tensor(out=t2sb[:], in0=t2p[:], in1=b2sb[:, :C],
                                            op=mybir.AluOpType.add)
                    nc.sync.dma_start(out=t2k[w * P:(w + 1) * P, :], in_=t2sb[:])
                    if w in ch_end:
                        ch = ch_end[w]
                        s0, sz = cfg["C2S"][ch], cfg["C2Z"][ch]
                        nc.gpsimd.collective_compute(
                            "AllGather", mybir.AluOpType.bypass, replica_groups=rg,
                            ins=[t2k[s0:s0 + sz, :]],
                            outs=[tb2[int(cfg["BASE2"][ch]):int(cfg["BASE2"][ch]) + N_CORES * sz, :]],
                        )

            # --- phase 3: L2 gather+reduce + log_softmax
            for w in range(NW):
                dw = int(D[w])
                c0 = int(woff[w])
                gts = []
                for s in range(dw):
                    gt = gpl.tile([P, C], dt.bfloat16, tag="gt2")
                    nc.gpsimd.indirect_dma_start(
                        out=gt[:], out_offset=None,
                        in_=tb2[:],
                        in_offset=bass.IndirectOffsetOnAxis(
                            ap=ix2sb[:, c0 + s:c0 + s + 1], axis=0),
                    )
                    gts.append(gt)
                red = ps.tile([P, C], dt.float32, tag="red")
                for s in range(dw):
                    nc.tensor.matmul(
                        out=red[:, :], lhsT=ident[:],
                        rhs=gts[s][:],
                        start=(s == 0), stop=(s == dw - 1),
                    )
                if True:
                    # log_softmax over classes
                    nm = sp.tile([P, 1], dt.float32, tag="nm")
                    nc.vector.tensor_reduce(
                        out=nm[:], in_=red[:], axis=mybir.AxisListType.X,
                        op=mybir.AluOpType.max, negate=True,
                    )
                    sc = sp.tile([P, C], dt.float32, tag="sc")
                    ssum = sp.tile([P, 1], dt.float32, tag="ssum")
                    nc.scalar.activation(
                        sc[:], red[:], mybir.ActivationFunctionType.Exp,
                        bias=nm[:], accum_out=ssum[:],
                    )
                    ls = sp.tile([P, 1], dt.float32, tag="ls")
                    nc.scalar.activation(ls[:], ssum[:], mybir.ActivationFunctionType.Ln)
                    ob = sp.tile([P, C], dt.float32, tag="ob")
                    nc.vector.tensor_scalar(
                        out=ob[:], in0=red[:], scalar1=nm[:], scalar2=ls[:],
                        op0=mybir.AluOpType.add, op1=mybir.AluOpType.subtract,
                    )
                    nc.sync.dma_start(out=outp[w * P:(w + 1) * P, :], in_=ob[:])

    nc.compile()
    return nc


# ---------------------------------------------------------------- entry point

LAST_RESULT = {}


def _run(cfg, x, edge_index, W1, b1, W2, b2, trace=False):
    from concourse.bass_utils import run_bass_kernel_spmd

    sched, in_maps, perms = host_prep(cfg, x, edge_index, W1, b1, W2, b2)
    nc = build_program(cfg, sched)
    res = run_bass_kernel_spmd(
        nc, in_maps, list(range(N_CORES)), trace=trace,
    )
    LAST_RESULT["exec_time_ns"] = res.exec_time_ns
    LAST_RESULT["mean_exec_time_ns"] = res.mean_exec_time_ns
    LAST_RESULT["trace"] = res.instructions_and_trace
    LAST_RESULT["profile_json"] = res.profile_json
    N, NP, C = cfg["N"], cfg["NP"], cfg["C"]
    full = np.empty((N, C), dtype=np.float32)
    for k in range(N_CORES):
        outk = np.asarray(res.results[k]["out"], dtype=np.float32)
        blk = full[k * NP:(k + 1) * NP]
        blk[perms[k]] = outk[:NP]
    return full


def kernel(x, edge_index, W1, b1, W2, b2):
    trace = bool(int(os.environ.get("GNN_TRACE", "0")))
    return _run(FULL_CFG, x, edge_index, W1, b1, W2, b2, trace=trace)

